# revision 1
# baseline (speedup 1.0000x reference)
"""Trainium2 Bass kernel for nn_Attention (8-head attention + positional-decay
branch), SPMD across 8 NeuronCores.

Sharding: data-parallel over batch x tensor-parallel over heads.
  core c: batch b = c//4, heads {2*(c%4), 2*(c%4)+1}  (2 "units" per core)
Each core computes, for its two heads:
  qkvt projection, softmax attention (out1), positional-decay attention
  (out2, banded: exp(-|i-j|/e) is < 3e-21 beyond |i-j|=128), and the out2
  half of to_out. The out1 half of to_out plus the softmax normalization
  (a per-free-dim-column broadcast no engine does cheaply) and the
  cross-head/batch reduction happen on host from per-core partials.

All matmuls run in float32r (full PE rate; fp32 is 4x slower).
"""

import sys

sys.path.insert(0, "/opt/trn_rl_repo")

import numpy as np

import concourse.bass as bass
import concourse.tile as tile
from concourse import bacc, mybir
from concourse.bass_utils import run_bass_kernel_spmd

F32 = mybir.dt.float32
F32R = mybir.dt.float32r
EXP = mybir.ActivationFunctionType.Exp

N = 2048          # sequence length
DIM = 512         # model dim
DH = 64           # head dim
B = 2             # batch
KT = 4            # dim // 128 contraction tiles
NB = 4            # n // 512
NI = 16           # n // 128
ICH = 2           # n // 1024 (i-chunks for the attention loop)
NCORES = 8


def build_program(reps: int = 1) -> bass.Bass:
    # Bacc (not raw Bass): its compile() pass moves matmul waits to
    # ldweights and splits excess waits into EventSemaphore instructions,
    # which walrus codegen's per-instruction wait-slot limits require.
    nc = bacc.Bacc(None)

    xt_d = nc.declare_dram_parameter("xt", [KT, 128, N], F32R, False)
    wq_d = nc.declare_dram_parameter("wq", [KT, 128, 128], F32R, False)
    wk_d = nc.declare_dram_parameter("wk", [KT, 128, 128], F32R, False)
    wvt_d = nc.declare_dram_parameter("wvt", [KT, 128, 256], F32R, False)
    gb_d = nc.declare_dram_parameter("gb", [6, 128, 512], F32R, False)
    rs_d = nc.declare_dram_parameter("rsinv", [128, NI], F32, False)
    lns_d = nc.declare_dram_parameter("lns", [128, NI], F32, False)
    w2t_d = nc.declare_dram_parameter("w2t", [64, 1024], F32R, False)
    o1t0_d = nc.declare_dram_parameter("o1t0", [65, N], F32, isOutput=True)
    o1t1_d = nc.declare_dram_parameter("o1t1", [65, N], F32, isOutput=True)
    f2_d = nc.declare_dram_parameter("f2", [NI, 128, 512], F32, isOutput=True)

    with tile.TileContext(nc) as tc:
        with (
            tc.tile_pool(name="const", bufs=1) as cp,
            tc.tile_pool(name="attn", bufs=12) as apool,
            tc.tile_pool(name="fout", bufs=3) as fpool,
            tc.tile_pool(name="psum", bufs=1, space="PSUM") as pp,
        ):
            for _rep in range(reps):
                # ---- resident SBUF tensors ----
                xt_sb = cp.tile([128, KT, N], F32R, name="xt_sb")
                wq_sb = cp.tile([128, KT, 128], F32R, name="wq_sb")
                wk_sb = cp.tile([128, KT, 128], F32R, name="wk_sb")
                wvt_sb = cp.tile([128, KT, 256], F32R, name="wvt_sb")
                g_sb = cp.tile([128, 6, 512], F32R, name="g_sb")
                rs_sb = cp.tile([128, NI], F32, name="rs_sb")
                lns_sb = cp.tile([128, NI], F32, name="lns_sb")
                w2t_sb = cp.tile([64, 1024], F32R, name="w2t_sb")
                qT = cp.tile([128, N], F32R, name="qT")
                kT = cp.tile([128, N], F32R, name="kT")
                # per j-block 128 (all 1/s(j)-scaled): cols
                # [t0' 0:64 | t1' 64:128 | v0' 128:192 | 1/s 192 |
                #  v1' 193:257 | pad]; exp carries bias ln(s_j) so the
                # net softmax weights and denominators are exact.
                vt_sb = cp.tile([128, NI, 260], F32R, name="vt_sb")
                o1sb = [
                    cp.tile([65, N], F32, name=f"o1sb{u}") for u in range(2)
                ]
                o2sb = [
                    cp.tile([64, N], F32R, name=f"o2sb{u}") for u in range(2)
                ]

                # warm the ACT exp table at t~0 (the PSEUDO table load
                # costs ~1.3us and would otherwise sit right before the
                # first real exp on the critical path)
                warm = cp.tile([1, 8], F32, name="warm")
                nc.vector.memset(warm[:], 0.0)
                nc.scalar.activation(warm[:], warm[:], EXP)

                # ---- input DMAs (critical-path first) ----
                # xt split by column-block: the first qk chunks need only
                # columns 0:1024, so they start ~6us earlier than with
                # whole-kt xt transfers.
                for kt in range(KT):
                    nc.sync.dma_start(out=wk_sb[:, kt, :], in_=wk_d[kt])
                    nc.sync.dma_start(out=wq_sb[:, kt, :], in_=wq_d[kt])
                for c4 in range(NB):
                    for kt in range(KT):
                        nc.sync.dma_start(
                            out=xt_sb[:, kt, c4 * 512:(c4 + 1) * 512],
                            in_=xt_d[kt, :, c4 * 512:(c4 + 1) * 512])
                for kt in range(KT):
                    nc.sync.dma_start(out=wvt_sb[:, kt, :], in_=wvt_d[kt])
                nc.sync.dma_start(out=rs_sb[:], in_=rs_d[:])
                nc.sync.dma_start(out=lns_sb[:], in_=lns_d[:])
                for gi in range(6):
                    nc.sync.dma_start(out=g_sb[:, gi, :], in_=gb_d[gi])
                nc.sync.dma_start(out=w2t_sb[:], in_=w2t_d[:])

                # PSUM budget is 8 banks total: four static 2-bank
                # tags (A,B = S^T tiles; C,D = out1 accumulators), shared by
                # the other phases (qk/vt chunks ride C/D before the out1
                # accumulators exist; out2/F2 ride A/B after the last exp).
                AB = ("psA", "psB")
                CD = ("psC", "psD")
                RA = 6          # dots/exp run-ahead (in jt) before vt is done

                def emit_qk_chunk(wsb, dst, c4, tag):
                    ps = pp.tile([128, 512], F32, tag=tag, bufs=1,
                                 name="qk_ps")
                    for kt in range(KT):
                        nc.tensor.matmul(
                            ps,
                            lhsT=wsb[:, kt, :],
                            rhs=xt_sb[:, kt, c4 * 512:(c4 + 1) * 512],
                            start=(kt == 0),
                            stop=(kt == KT - 1),
                        )
                    nc.vector.tensor_copy(
                        dst[:, c4 * 512:(c4 + 1) * 512], ps[:])

                def emit_vt(ib, tag):
                    ps = pp.tile([128, 256], F32, tag=tag, bufs=1,
                                 name="vt_ps")
                    for kt in range(KT):
                        nc.tensor.matmul(
                            ps,
                            lhsT=xt_sb[:, kt, ib * 128:(ib + 1) * 128],
                            rhs=wvt_sb[:, kt, :],
                            start=(kt == 0),
                            stop=(kt == KT - 1),
                        )
                    # psum cols: [t0 t1 v0 v1]; everything scaled by 1/s
                    nc.vector.tensor_scalar_mul(
                        vt_sb[:, ib, 0:192], ps[:, 0:192], rs_sb[:, ib:ib + 1])
                    nc.vector.tensor_scalar_mul(
                        vt_sb[:, ib, 193:257], ps[:, 192:256],
                        rs_sb[:, ib:ib + 1])

                def emit_dots_exp(ich, jt, u):
                    st = pp.tile([128, 1024], F32, tag=AB[u], bufs=1,
                                 name=f"st_ps{u}")
                    for hf in range(2):
                        c0 = ich * 1024 + hf * 512
                        nc.tensor.matmul(
                            st[:, hf * 512:(hf + 1) * 512],
                            lhsT=kT[u * 64:(u + 1) * 64,
                                    jt * 128:(jt + 1) * 128],
                            rhs=qT[u * 64:(u + 1) * 64, c0:c0 + 512],
                            start=True,
                            stop=True,
                            tile_position=(u * 64, 0),
                        )
                    at = apool.tile([128, 1024], F32R, tag="attnT", name="at")
                    nc.scalar.activation(at[:], st[:], EXP,
                                         bias=lns_sb[:, jt:jt + 1])
                    return at

                def emit_out1(o1ps, jt, u, at):
                    # lhsT u0 [v0|ones] -> psum rows 0:64 out1, row 64 = r
                    #      u1 [ones|v1] -> psum row 0 = r, rows 1:65 out1
                    for hf in range(2):
                        nc.tensor.matmul(
                            o1ps[u][:, hf * 512:(hf + 1) * 512],
                            lhsT=vt_sb[:, jt, 128 + u * 64:193 + u * 64],
                            rhs=at[:, hf * 512:(hf + 1) * 512],
                            start=(jt == 0),
                            stop=(jt == NI - 1),
                            skip_group_check=True,
                        )

                # ---- prologue: qk chunks interleaved with early dots ----
                emit_qk_chunk(wk_sb, kT, 0, CD[0])
                emit_qk_chunk(wk_sb, kT, 1, CD[1])
                emit_qk_chunk(wq_sb, qT, 0, CD[0])
                emit_qk_chunk(wq_sb, qT, 1, CD[1])
                ats = {}
                for jt in range(2):
                    for u in range(2):
                        ats[(jt, u)] = emit_dots_exp(0, jt, u)
                emit_qk_chunk(wk_sb, kT, 2, CD[0])
                emit_qk_chunk(wq_sb, qT, 2, CD[1])
                for jt in range(2, 4):
                    for u in range(2):
                        ats[(jt, u)] = emit_dots_exp(0, jt, u)
                emit_qk_chunk(wk_sb, kT, 3, CD[0])
                emit_qk_chunk(wq_sb, qT, 3, CD[1])
                for jt in range(4, RA):
                    for u in range(2):
                        ats[(jt, u)] = emit_dots_exp(0, jt, u)
                # column 192 = 1/s(j): the "denominator" lhsT column.
                # exp carries bias ln(s_j), so rows of exp(S^T) are scaled
                # by s_j; v and t are scaled by 1/s_j (fused tensor_scalar
                # evacs) and the 1/s column recovers the plain softmax
                # denominator sum.
                nc.vector.tensor_copy(vt_sb[:, :, 192:193], rs_sb[:])
                for ib in range(NI):
                    emit_vt(ib, CD[ib % 2])

                # ---- attention main loop (ich0 drains the run-ahead
                #      backlog one jt per iteration to keep ACT fed) ----
                for ich in range(ICH):
                    o1ps = [
                        pp.tile([65, 1024], F32, tag=CD[u], bufs=1,
                                name=f"o1_ps{u}")
                        for u in range(2)
                    ]
                    backlog = list(range(RA)) if ich == 0 else []
                    start_jt = RA if ich == 0 else 0
                    for jt in range(start_jt, NI):
                        cur = [emit_dots_exp(ich, jt, u) for u in range(2)]
                        if backlog and ((jt - start_jt) % 2 == 0
                                        or jt >= NI - 2):
                            bj = backlog.pop(0)
                            for u in range(2):
                                emit_out1(o1ps, bj, u, ats.pop((bj, u)))
                        for u in range(2):
                            emit_out1(o1ps, jt, u, cur[u])
                    while backlog:
                        bj = backlog.pop(0)
                        for u in range(2):
                            emit_out1(o1ps, bj, u, ats.pop((bj, u)))
                    for u in range(2):
                        nc.vector.tensor_copy(
                            o1sb[u][:, ich * 1024:(ich + 1) * 1024],
                            o1ps[u][:])
                    if ich == ICH - 1:
                        nc.sync.dma_start(out=o1t0_d[:], in_=o1sb[0][:])
                        nc.sync.dma_start(out=o1t1_d[:], in_=o1sb[1][:])

                # ---- out2 (banded, 256-wide i-chunks) + F2 ----
                # a2^T block (jt, chunk c) = g(|f - p - d|), d = jt*128-c*256
                # in {-128, 0, 128, 256} -> slices of the resident g blocks.
                # st/o1 psum tags are all free here; rotate through all four
                # so the out2 -> evac -> F2 -> evac -> DMA chain pipelines.
                tags4 = AB + CD
                tagn = [0]

                def next_tag():
                    tagn[0] += 1
                    return tags4[tagn[0] % 4]

                for c in range(8):
                    for u in range(2):
                        ps = pp.tile([64, 256], F32, tag=next_tag(), bufs=1,
                                     name="o2_ps")
                        jts = [jt for jt in range(2 * c - 1, 2 * c + 3)
                               if 0 <= jt < NI]
                        for idx, jt in enumerate(jts):
                            gi = jt - 2 * c + 1
                            nc.tensor.matmul(
                                ps,
                                lhsT=vt_sb[:, jt, u * 64:(u + 1) * 64],
                                rhs=g_sb[:, gi, 0:256],
                                start=(idx == 0),
                                stop=(idx == len(jts) - 1),
                            )
                        if (c + u) % 2 == 0:
                            nc.vector.tensor_copy(
                                o2sb[u][:, c * 256:(c + 1) * 256], ps[:])
                        else:
                            nc.scalar.copy(
                                o2sb[u][:, c * 256:(c + 1) * 256], ps[:])
                    if c % 2 == 1:
                        for ib in range(2 * c - 2, 2 * c + 2):
                            fps = pp.tile([128, 512], F32, tag=next_tag(),
                                          bufs=1, name="f2_ps")
                            for u in range(2):
                                nc.tensor.matmul(
                                    fps,
                                    lhsT=o2sb[u][:, ib * 128:(ib + 1) * 128],
                                    rhs=w2t_sb[:, u * 512:(u + 1) * 512],
                                    start=(u == 0),
                                    stop=(u == 1),
                                )
                            f2t = fpool.tile([128, 512], F32, tag="f2sb",
                                             name="f2t")
                            if ib % 2 == 0:
                                nc.vector.tensor_copy(f2t[:], fps[:])
                            else:
                                nc.scalar.copy(f2t[:], fps[:])
                            nc.sync.dma_start(out=f2_d[ib], in_=f2t[:])



    nc.finalize()
    return nc


_PROGRAM = None


def _get_program():
    global _PROGRAM
    if _PROGRAM is None:
        _PROGRAM = build_program()
    return _PROGRAM


def _host_tables():
    d = np.arange(N, dtype=np.float64)
    g = np.exp(-d / np.e)
    cum = np.cumsum(g)
    j = np.arange(N)
    s = cum[j] + cum[N - 1 - j] - g[0]          # s[j] = sum_k exp(-|j-k|/e)
    rsinv = (1.0 / s).reshape(NI, 128).T.astype(np.float32)  # [128, NI]
    lns = np.log(s).reshape(NI, 128).T.astype(np.float32)    # [128, NI]
    gi = np.arange(6)[:, None, None]
    p = np.arange(128)[None, :, None]
    f = np.arange(512)[None, None, :]
    gb = np.exp(-np.abs(f - p - (gi - 1) * 128) / np.e).astype(np.float32)
    return (np.ascontiguousarray(rsinv), np.ascontiguousarray(lns),
            np.ascontiguousarray(gb))


def make_in_maps(x, w_qkv, w_out, b_out):
    x = np.asarray(x, np.float32)
    w_qkv = np.asarray(w_qkv, np.float32)
    w_out = np.asarray(w_out, np.float32)
    rsinv, lns, gb = _host_tables()
    scale = float(DH) ** -0.5

    wq_full = w_qkv[0:512]
    wk_full = w_qkv[512:1024]
    wv_full = w_qkv[1024:1536]
    wt_full = w_qkv[1536:2048]

    def heads(c):
        h0 = 2 * (c % 4)
        return h0, h0 + 1

    in_maps = []
    for c in range(NCORES):
        b = c // 4
        h0, h1 = heads(c)
        xt = np.ascontiguousarray(x[b].T.reshape(KT, 128, N))

        def pack2(wfull, scl=1.0):
            wt_ = np.concatenate(
                [wfull[h0 * 64:(h0 + 1) * 64].T * scl,
                 wfull[h1 * 64:(h1 + 1) * 64].T * scl], axis=1)
            return np.ascontiguousarray(
                wt_.reshape(KT, 128, 128).astype(np.float32))

        wq = pack2(wq_full, scale)
        wk = pack2(wk_full)
        wvt_ = np.concatenate(
            [wt_full[h0 * 64:(h0 + 1) * 64].T,
             wt_full[h1 * 64:(h1 + 1) * 64].T,
             wv_full[h0 * 64:(h0 + 1) * 64].T,
             wv_full[h1 * 64:(h1 + 1) * 64].T], axis=1)
        wvt = np.ascontiguousarray(
            wvt_.reshape(KT, 128, 256).astype(np.float32))
        w2t = np.ascontiguousarray(np.concatenate(
            [w_out[:, h0 * 128 + 64:(h0 + 1) * 128].T,
             w_out[:, h1 * 128 + 64:(h1 + 1) * 128].T],
            axis=1).astype(np.float32))
        in_maps.append({
            "xt": xt, "wq": wq, "wk": wk, "wvt": wvt,
            "gb": gb, "rsinv": rsinv, "lns": lns, "w2t": w2t,
        })
    return in_maps


def _heads(c):
    h0 = 2 * (c % 4)
    return h0, h0 + 1


def combine_outputs(results, w_out, b_out):
    """Host-side unshard: per-core partials -> full [B, N, DIM] output."""
    w_out = np.asarray(w_out, np.float32)
    b_out = np.asarray(b_out, np.float32)
    out = np.zeros((B, N, DIM), np.float64)
    for c in range(NCORES):
        r = results[c]
        b = c // 4
        h0, h1 = _heads(c)
        o1_0 = r["o1t0"][0:64].T.astype(np.float64)   # [N, 64]
        r0 = r["o1t0"][64].astype(np.float64)
        r1 = r["o1t1"][0].astype(np.float64)
        o1_1 = r["o1t1"][1:65].T.astype(np.float64)
        f2 = r["f2"].reshape(N, 512).astype(np.float64)
        w1_0 = w_out[:, h0 * 128:h0 * 128 + 64].T.astype(np.float64)
        w1_1 = w_out[:, h1 * 128:h1 * 128 + 64].T.astype(np.float64)
        part = (o1_0 / r0[:, None]) @ w1_0 + (o1_1 / r1[:, None]) @ w1_1 + f2
        out[b] += part
    out += b_out[None, None, :].astype(np.float64)
    return out.astype(np.float32)


def kernel(x, w_qkv, w_out, b_out):
    nc = _get_program()
    in_maps = make_in_maps(x, w_qkv, w_out, b_out)
    res = run_bass_kernel_spmd(nc, in_maps, core_ids=list(range(NCORES)))
    return combine_outputs(res.results, w_out, b_out)


def kernel_profiled(x, w_qkv, w_out, b_out):
    # NTFF tracing is unavailable in this container (no antenv.axon_hooks);
    # run untraced and let the caller time executions.
    out = kernel(x, w_qkv, w_out, b_out)
    return out, None



# revision 14
# speedup vs baseline: 1.1927x; 1.1927x over previous
"""Trainium2 Bass kernel for nn_Attention (8-head attention + positional-decay
branch), SPMD across 8 NeuronCores.

Sharding: data-parallel over batch x tensor-parallel over heads.
  core c: batch b = c//4, heads {2*(c%4), 2*(c%4)+1}  (2 "units" per core)

v2 dataflow (per core):
  - bf16 projections (qkvt) and bf16 out2/F2 (positional branch; the decay
    matrix is head-independent so both heads share one 128-partition matmul).
  - fp8e4m3 + DoubleRow matmuls for dots and out1 (0.5 PE cycles/row).
    q/k are evacuated to fp8 and remapped [dh, n] -> [dh%32, dh//32, n] by
    SBUF->SBUF DMA so the 64-deep head contraction becomes 32 partitions x
    2 DoubleRow planes.
  - at = exp(S + C) with a global constant bias C (range-fit for fp8).  Any
    scale on the attention weights cancels in the host-side o1/r division,
    so the softmax stays exact up to dtype noise.
  - exp split across two engines: ACT (table exp -> fp8 at, DoubleRow out1)
    and, for the pairs in M_DVE, DVE (Schraudolph bit-trick exp:
    int16(x*A+B) bitcast to bf16, ~3% max rel err; bf16 out1).
  - softmax denominator r rides out1 as stationary column 64 (constant d).
  - o1 and F2 PSUM tiles are DMA'd straight to HBM (no evacuation pass).
  - host: out1 = o1[0:64]/o1[64], out1-half of to_out, cross-core sum.
"""

import sys

sys.path.insert(0, "/opt/trn_rl_repo")

import numpy as np
import ml_dtypes

import concourse.bass as bass
import concourse.tile as tile
from concourse import bacc, mybir
from concourse.bass_utils import run_bass_kernel_spmd

F32 = mybir.dt.float32
F16 = mybir.dt.float16
BF16 = mybir.dt.bfloat16
F8 = mybir.dt.float8e4
I16 = mybir.dt.int16
EXP = mybir.ActivationFunctionType.Exp
DR = mybir.MatmulPerfMode.DoubleRow

N = 2048          # sequence length
DH = 64           # head dim
B = 2             # batch
KT = 4            # dim // 128 contraction tiles
NI = 16           # n // 128 j-tiles
NP = 8            # n // 256 j-tile pairs
NCORES = 8

# (u, pair) units whose exp runs on DVE (Schraudolph/bf16 route)
M_DVE = frozenset({(0, 2), (1, 2), (0, 5), (1, 5), (0, 7)})

# global exp bias: at = exp(S + C_BIAS) <= exp(SMAX + C_BIAS) ~ 170 < 240
C_BIAS = float(np.log(170.0) - 6.2)
# Schraudolph bf16-bitcast exp: i16 = int(x*A + B); +0.5 centers the
# truncating float->int conversion.
SCH_A = 128.0 / np.log(2.0)
SCH_B = 127.0 * 128.0 - 0.0301975 * 128.0 + 0.5 + SCH_A * C_BIAS
VSCALE = 0.25     # v'' = v * VSCALE; d column = VSCALE (cancels in o1/r)


def build_program() -> bass.Bass:
    nc = bacc.Bacc(None)

    xt_d = nc.declare_dram_parameter("xt", [KT, 128, N], BF16, False)
    wq_d = nc.declare_dram_parameter("wq", [KT, 128, 128], BF16, False)
    wk_d = nc.declare_dram_parameter("wk", [KT, 128, 128], BF16, False)
    wvt_d = nc.declare_dram_parameter("wvt", [KT, 128, 256], BF16, False)
    gb_d = nc.declare_dram_parameter("gb", [4, 128, 256], BF16, False)
    w2s_d = nc.declare_dram_parameter("w2s", [128, 512], BF16, False)
    rst_d = nc.declare_dram_parameter("rst", [128, NI], F32, False)
    o1t0_d = nc.declare_dram_parameter("o1t0", [65, N], F16, isOutput=True)
    o1t1_d = nc.declare_dram_parameter("o1t1", [65, N], F16, isOutput=True)
    f2_d = nc.declare_dram_parameter("f2", [NI, 128, 512], F16, isOutput=True)
    o1t_d = [o1t0_d, o1t1_d]

    with tile.TileContext(nc) as tc:
        with (
            tc.tile_pool(name="const", bufs=1) as cp,
            tc.tile_pool(name="at8p", bufs=10) as ap8,
            tc.tile_pool(name="atip", bufs=6) as api,
            tc.tile_pool(name="fout", bufs=4) as fpool,
            tc.tile_pool(name="psum", bufs=1, space="PSUM") as pp,
        ):
            # ---- resident SBUF tensors ----
            xt_sb = cp.tile([128, KT, N], BF16, name="xt_sb")
            wq_sb = cp.tile([128, KT, 128], BF16, name="wq_sb")
            wk_sb = cp.tile([128, KT, 128], BF16, name="wk_sb")
            wvt_sb = cp.tile([128, KT, 256], BF16, name="wvt_sb")
            g_sb = cp.tile([128, 4, 256], BF16, name="g_sb")
            w2s_sb = cp.tile([128, 512], BF16, name="w2s_sb")
            rst_sb = cp.tile([128, NI], F32, name="rst_sb")
            q8tmp = cp.tile([128, N], F8, name="q8tmp")
            k8tmp = cp.tile([128, N], F8, name="k8tmp")
            q8 = cp.tile([64, 2, N], F8, name="q8")
            k8 = cp.tile([64, 2, N], F8, name="k8")
            # v''/d stationary, fp8 DR layout: [p=j, u, jtile, 128] with
            # cols 0:64 = v*VSCALE, col 64 = VSCALE
            vd8 = cp.tile([128, 2, NI, 128], F8, name="vd8")
            # bf16 copy for the DVE-exp route
            vd16 = cp.tile([128, 2, NI, 66], BF16, name="vd16")
            # t' = t/s_j for out2 (cols 0:64 head u0, 64:128 head u1)
            vtt = cp.tile([128, NI, 128], BF16, name="vtt")
            o2sb = cp.tile([128, N], BF16, name="o2sb")
            o1sb = [cp.tile([65, N], F16, name=f"o1sb{u}") for u in range(2)]

            # warm the ACT exp table off the critical path
            warm = cp.tile([1, 8], F32, name="warm")
            nc.vector.memset(warm[:], 0.0)
            nc.scalar.activation(warm[:], warm[:], EXP)
            # denominator columns (vd16's col 64 comes from the gpsimd copy)
            nc.vector.memset(vd8[:, :, :, 64:65], VSCALE)
            # per-partition exp bias column (scalar consts need an AP)
            cbias = cp.tile([128, 1], F32, name="cbias")
            nc.vector.memset(cbias[:], C_BIAS)

            # ---- input DMAs (critical-path first) ----
            for kt in range(KT):
                nc.sync.dma_start(out=wk_sb[:, kt, :], in_=wk_d[kt])
                nc.sync.dma_start(out=wq_sb[:, kt, :], in_=wq_d[kt])
            for c4 in range(4):
                for kt in range(KT):
                    nc.sync.dma_start(
                        out=xt_sb[:, kt, c4 * 512:(c4 + 1) * 512],
                        in_=xt_d[kt, :, c4 * 512:(c4 + 1) * 512])
            for kt in range(KT):
                nc.sync.dma_start(out=wvt_sb[:, kt, :], in_=wvt_d[kt])
            nc.sync.dma_start(out=rst_sb[:], in_=rst_d[:])
            for gi in range(4):
                nc.sync.dma_start(out=g_sb[:, gi, :], in_=gb_d[gi])
            nc.sync.dma_start(out=w2s_sb[:], in_=w2s_d[:])

            # ---- emit helpers.  PSUM tags: A0,A1,B0,B1 1 bank; E,F 2 ----
            def emit_qk_chunk(wsb, tmp, c4, tag):
                ps = pp.tile([128, 512], F32, tag=tag, bufs=1, name="qk_ps")
                for kt in range(KT):
                    nc.tensor.matmul(
                        ps, lhsT=wsb[:, kt, :],
                        rhs=xt_sb[:, kt, c4 * 512:(c4 + 1) * 512],
                        start=(kt == 0), stop=(kt == KT - 1))
                nc.vector.tensor_copy(tmp[:, c4 * 512:(c4 + 1) * 512], ps[:])

            def emit_remap(tmp, dst, c4):
                # [dh(128), 512] fp8 -> [32u + dh%32, dh//32, 512]
                cs = slice(c4 * 512, (c4 + 1) * 512)
                for u in range(2):
                    for pl in range(2):
                        nc.sync.dma_start(
                            out=dst[u * 32:(u + 1) * 32, pl, cs],
                            in_=tmp[u * 64 + pl * 32:u * 64 + (pl + 1) * 32,
                                    cs])

            def emit_vt(ib, tag):
                ps = pp.tile([128, 256], F32, tag=tag, bufs=1, name="vt_ps")
                for kt in range(KT):
                    nc.tensor.matmul(
                        ps, lhsT=xt_sb[:, kt, ib * 128:(ib + 1) * 128],
                        rhs=wvt_sb[:, kt, :],
                        start=(kt == 0), stop=(kt == KT - 1))
                nc.vector.tensor_scalar_mul(
                    vtt[:, ib, :], ps[:, 0:128], rst_sb[:, ib:ib + 1])
                vsrc = ps[:, 128:256].rearrange("p (u f) -> p u f", u=2)
                nc.vector.tensor_copy(vd8[:, :, ib, 0:64], vsrc)

            at_tiles = {}
            ef_rot = [0]

            def emit_fill(ich, p, u, h):
                """dots psum fill [128 j(2 planes), 512 i] + exp."""
                ef_rot[0] ^= 1
                ps = pp.tile([128, 2, 512], F32,
                             tag=("E" if ef_rot[0] else "Fq"), bufs=1,
                             name="st_ps")
                i0 = ich * 1024 + h * 512
                for pl in range(2):
                    for ic in range(2):
                        nc.tensor.matmul(
                            ps[:, pl, ic * 256:(ic + 1) * 256],
                            lhsT=k8[u * 32:(u + 1) * 32, :,
                                    (p * 2 + pl) * 128:(p * 2 + pl + 1) * 128],
                            rhs=q8[u * 32:(u + 1) * 32, :,
                                   i0 + ic * 256:i0 + (ic + 1) * 256],
                            start=(ic == 0), stop=(ic == 1),
                            perf_mode=DR, skip_group_check=True)
                key = (ich, u, p)
                if (u, p) in M_DVE:
                    if key not in at_tiles:
                        at_tiles[key] = api.tile([128, 2, 1024], I16,
                                                 tag="ati", name="ati")
                    at = at_tiles[key]
                    nc.vector.tensor_scalar(
                        out=at[:, :, h * 512:(h + 1) * 512], in0=ps[:],
                        scalar1=SCH_A, scalar2=SCH_B,
                        op0=mybir.AluOpType.mult, op1=mybir.AluOpType.add)
                else:
                    if key not in at_tiles:
                        at_tiles[key] = ap8.tile([128, 2, 1024], F8,
                                                 tag="at8", name="at8")
                    at = at_tiles[key]
                    nc.scalar.activation(
                        at[:, :, h * 512:(h + 1) * 512], ps[:], EXP,
                        bias=cbias[:])

            o1ps = {}

            def emit_out1(ich, u, p):
                key = (ich, u, p)
                at = at_tiles.pop(key)
                tags = ("A0", "A1") if u == 0 else ("B0", "B1")
                for hb in range(2):
                    if (u, ich, hb) not in o1ps:
                        o1ps[(u, ich, hb)] = pp.tile(
                            [65, 512], F32, tag=tags[hb], bufs=1,
                            name=f"o1_ps{u}{hb}")
                if (u, p) in M_DVE:
                    for pl in range(2):
                        for hb in range(2):
                            nc.tensor.matmul(
                                o1ps[(u, ich, hb)],
                                lhsT=vd16[:, u, p * 2 + pl, 0:65],
                                rhs=at[:, pl,
                                       hb * 512:(hb + 1) * 512].bitcast(BF16),
                                start=False, stop=(p == NP - 1 and pl == 1),
                                skip_group_check=True)
                else:
                    for hb in range(2):
                        for ic in range(2):
                            nc.tensor.matmul(
                                o1ps[(u, ich, hb)][:, ic * 256:(ic + 1) * 256],
                                lhsT=vd8[:, u, p * 2:p * 2 + 2, 0:65],
                                rhs=at[:, :,
                                       hb * 512 + ic * 256:
                                       hb * 512 + (ic + 1) * 256],
                                start=(p == 0 and ic == 0),
                                stop=(p == NP - 1),
                                perf_mode=DR, skip_group_check=True)

            def emit_o1_dma(ich, u):
                for hb in range(2):
                    ps = o1ps.pop((u, ich, hb))
                    dst = o1sb[u][:, ich * 1024 + hb * 512:
                                  ich * 1024 + (hb + 1) * 512]
                    if hb == 0:
                        nc.vector.tensor_copy(dst, ps[:])
                    else:
                        nc.scalar.copy(dst, ps[:])
                nc.sync.dma_start(
                    out=o1t_d[u][:, ich * 1024:(ich + 1) * 1024],
                    in_=o1sb[u][:, ich * 1024:(ich + 1) * 1024])

            def emit_out2(c, tag):
                ps = pp.tile([128, 256], F32, tag=tag, bufs=1, name="o2_ps")
                jts = [jt for jt in range(2 * c - 1, 2 * c + 3)
                       if 0 <= jt < NI]
                for idx, jt in enumerate(jts):
                    nc.tensor.matmul(
                        ps, lhsT=vtt[:, jt, :],
                        rhs=g_sb[:, jt - 2 * c + 1, :],
                        start=(idx == 0), stop=(idx == len(jts) - 1))
                nc.vector.tensor_copy(o2sb[:, c * 256:(c + 1) * 256], ps[:])

            def emit_f2(ib, tag):
                ps = pp.tile([128, 512], F32, tag=tag, bufs=1, name="f2_ps")
                nc.tensor.matmul(
                    ps, lhsT=o2sb[:, ib * 128:(ib + 1) * 128],
                    rhs=w2s_sb[:], start=True, stop=True)
                f2t = fpool.tile([128, 512], F16, tag="f2sb", name="f2t")
                if ib % 2 == 0:
                    nc.vector.tensor_copy(f2t[:], ps[:])
                else:
                    nc.scalar.copy(f2t[:], ps[:])
                nc.sync.dma_start(out=f2_d[ib], in_=f2t[:])

            # ---- prologue: qk chunks -> fp8 -> remap; early dots fills ----
            emit_qk_chunk(wk_sb, k8tmp, 0, "A0")
            emit_remap(k8tmp, k8, 0)
            emit_qk_chunk(wq_sb, q8tmp, 0, "A1")
            emit_remap(q8tmp, q8, 0)
            emit_qk_chunk(wk_sb, k8tmp, 1, "A0")
            emit_remap(k8tmp, k8, 1)
            emit_qk_chunk(wq_sb, q8tmp, 1, "A1")
            emit_remap(q8tmp, q8, 1)
            for p in range(2):
                for u in range(2):
                    emit_fill(0, p, u, 0)
            emit_qk_chunk(wk_sb, k8tmp, 2, "A0")
            emit_remap(k8tmp, k8, 2)
            emit_qk_chunk(wq_sb, q8tmp, 2, "A1")
            emit_remap(q8tmp, q8, 2)
            for p in range(2):
                for u in range(2):
                    emit_fill(0, p, u, 1)
            emit_qk_chunk(wk_sb, k8tmp, 3, "A0")
            emit_remap(k8tmp, k8, 3)
            emit_qk_chunk(wq_sb, q8tmp, 3, "A1")
            emit_remap(q8tmp, q8, 3)

            # ---- main: remaining ich0 fills + vt + out2 + F2 ----
            b_rot = [0]

            def btag():
                b_rot[0] ^= 1
                return "B0" if b_rot[0] else "B1"

            for p in range(NP):
                fp = p + 2
                if fp < NP:
                    for h in range(2):
                        for u in range(2):
                            emit_fill(0, fp, u, h)
                emit_vt(2 * p, "A0")
                emit_vt(2 * p + 1, "A1")
                if p >= 1:
                    emit_out2(p - 1, btag())
            # bf16 stationary copy for the DVE-route out1 (Pool engine;
            # sources the fp8 values so both routes see the same v)
            nc.gpsimd.tensor_copy(vd16[:, :, :, 0:65], vd8[:, :, :, 0:65])
            emit_out2(NP - 1, btag())
            for ib in range(NI):
                emit_f2(ib, btag())
            # out1 for ich0 (after F2 so the A/B banks are clear; at tiles
            # buffered in SBUF meanwhile)
            for p in range(NP):
                for u in range(2):
                    emit_out1(0, u, p)
            for u in range(2):
                emit_o1_dma(0, u)
            # ---- ich1 fills + out1 ----
            for p in range(NP):
                for h in range(2):
                    for u in range(2):
                        emit_fill(1, p, u, h)
                if p >= 2:
                    for u in range(2):
                        emit_out1(1, u, p - 2)
            for p in range(NP - 2, NP):
                for u in range(2):
                    emit_out1(1, u, p)
            for u in range(2):
                emit_o1_dma(1, u)

    nc.finalize()
    return nc


_PROGRAM = None


def _get_program():
    global _PROGRAM
    if _PROGRAM is None:
        _PROGRAM = build_program()
    return _PROGRAM


def _host_tables():
    d = np.arange(N, dtype=np.float64)
    g = np.exp(-d / np.e)
    cum = np.cumsum(g)
    j = np.arange(N)
    s = cum[j] + cum[N - 1 - j] - g[0]        # s[j] = sum_k exp(-|j-k|/e)
    rst = np.ascontiguousarray(
        (1.0 / s).reshape(NI, 128).T.astype(np.float32))
    gi = np.arange(4)[:, None, None]
    p = np.arange(128)[None, :, None]
    f = np.arange(256)[None, None, :]
    gb = np.ascontiguousarray(
        np.exp(-np.abs(f - p - (gi - 1) * 128) / np.e)
        .astype(ml_dtypes.bfloat16))
    return rst, gb


_TABLES = None


def _tables():
    global _TABLES
    if _TABLES is None:
        _TABLES = _host_tables()
    return _TABLES


def make_in_maps(x, w_qkv, w_out, b_out):
    x = np.asarray(x, np.float32)
    w_qkv = np.asarray(w_qkv, np.float32)
    w_out = np.asarray(w_out, np.float32)
    rst, gb = _tables()
    qk_scale = float(DH) ** -0.25        # 1/sqrt(8) folded into both w's

    wq_full = w_qkv[0:512]
    wk_full = w_qkv[512:1024]
    wv_full = w_qkv[1024:1536]
    wt_full = w_qkv[1536:2048]

    bf = ml_dtypes.bfloat16
    in_maps = []
    for c in range(NCORES):
        b = c // 4
        h0 = 2 * (c % 4)
        h1 = h0 + 1

        xt = np.ascontiguousarray(x[b].T.reshape(KT, 128, N).astype(bf))

        def pack2(wfull, scl):
            wt_ = np.concatenate(
                [wfull[h0 * 64:(h0 + 1) * 64].T * scl,
                 wfull[h1 * 64:(h1 + 1) * 64].T * scl], axis=1)
            return np.ascontiguousarray(
                wt_.reshape(KT, 128, 128).astype(bf))

        wq = pack2(wq_full, qk_scale)
        wk = pack2(wk_full, qk_scale)
        wvt_ = np.concatenate(
            [wt_full[h0 * 64:(h0 + 1) * 64].T,
             wt_full[h1 * 64:(h1 + 1) * 64].T,
             wv_full[h0 * 64:(h0 + 1) * 64].T * VSCALE,
             wv_full[h1 * 64:(h1 + 1) * 64].T * VSCALE], axis=1)
        wvt = np.ascontiguousarray(wvt_.reshape(KT, 128, 256).astype(bf))
        w2s = np.ascontiguousarray(np.concatenate(
            [w_out[:, h0 * 128 + 64:(h0 + 1) * 128].T,
             w_out[:, h1 * 128 + 64:(h1 + 1) * 128].T],
            axis=0).astype(bf))
        in_maps.append({
            "xt": xt, "wq": wq, "wk": wk, "wvt": wvt,
            "gb": gb, "w2s": w2s, "rst": rst,
        })
    return in_maps


def combine_outputs(results, w_out, b_out):
    """Host-side unshard: per-core partials -> full [B, N, DIM] output."""
    w_out = np.asarray(w_out, np.float32)
    b_out = np.asarray(b_out, np.float32)
    out = np.zeros((B, N, 512), np.float64)
    for c in range(NCORES):
        r = results[c]
        b = c // 4
        h0 = 2 * (c % 4)
        part = r["f2"].reshape(N, 512).astype(np.float64)
        for u, h in ((0, h0), (1, h0 + 1)):
            o1 = r[f"o1t{u}"].astype(np.float64)
            o1v = (o1[0:64] / o1[64][None, :]).T            # [N, 64]
            w1 = w_out[:, h * 128:h * 128 + 64].T.astype(np.float64)
            part = part + o1v @ w1
        out[b] += part
    out += b_out[None, None, :].astype(np.float64)
    return out.astype(np.float32)


def kernel(x, w_qkv, w_out, b_out):
    nc = _get_program()
    in_maps = make_in_maps(x, w_qkv, w_out, b_out)
    res = run_bass_kernel_spmd(nc, in_maps, core_ids=list(range(NCORES)))
    return combine_outputs(res.results, w_out, b_out)


def kernel_profiled(x, w_qkv, w_out, b_out):
    out = kernel(x, w_qkv, w_out, b_out)
    return out, None


# revision 28
# speedup vs baseline: 1.3482x; 1.1304x over previous
"""Trainium2 Bass kernel for nn_Attention (8-head attention + positional-decay
branch), SPMD across 8 NeuronCores.

Sharding: data-parallel over batch x tensor-parallel over heads.
  core c: batch b = c//4, heads {2*(c%4), 2*(c%4)+1}  (2 "units" per core)

v2 dataflow (per core):
  - bf16 projections (qkvt) and bf16 out2/F2 (positional branch; the decay
    matrix is head-independent so both heads share one 128-partition matmul).
  - fp8e4m3 + DoubleRow matmuls for dots and out1 (0.5 PE cycles/row).
    q/k are evacuated to fp8 and remapped [dh, n] -> [dh%32, dh//32, n] by
    SBUF->SBUF DMA so the 64-deep head contraction becomes 32 partitions x
    2 DoubleRow planes.
  - at = exp(S + C) with a global constant bias C (range-fit for fp8).  Any
    scale on the attention weights cancels in the host-side o1/r division,
    so the softmax stays exact up to dtype noise.
  - exp split across two engines: ACT (table exp -> fp8 at, DoubleRow out1)
    and, for the pairs in M_DVE, DVE (Schraudolph bit-trick exp:
    int16(x*A+B) bitcast to bf16, ~3% max rel err; bf16 out1).
  - softmax denominator r rides out1 as stationary column 64 (constant d).
  - o1 and F2 PSUM tiles are DMA'd straight to HBM (no evacuation pass).
  - host: out1 = o1[0:64]/o1[64], out1-half of to_out, cross-core sum.
"""

import sys

sys.path.insert(0, "/opt/trn_rl_repo")

import numpy as np
import ml_dtypes

import concourse.bass as bass
import concourse.tile as tile
from concourse import bacc, mybir
from concourse.bass_utils import run_bass_kernel_spmd

F32 = mybir.dt.float32
F16 = mybir.dt.float16
BF16 = mybir.dt.bfloat16
F8 = mybir.dt.float8e4
I16 = mybir.dt.int16
EXP = mybir.ActivationFunctionType.Exp
DR = mybir.MatmulPerfMode.DoubleRow

N = 2048          # sequence length
DH = 64           # head dim
B = 2             # batch
KT = 4            # dim // 128 contraction tiles
NI = 16           # n // 128 j-tiles
NP = 8            # n // 256 j-tile pairs
NCORES = 8

# (ich, u, pair) units whose exp runs on DVE (Schraudolph/bf16 route)
M_DVE = frozenset({
    (i, u, p) for i in range(2) for u in range(2) for p in (2, 4)
} | {(0, 0, 6), (1, 0, 6), (0, 1, 6)})

# global exp bias: at = exp(S + C_BIAS) <= exp(SMAX + C_BIAS) ~ 170 < 240
C_BIAS = float(np.log(170.0) - 6.2)
# Schraudolph bf16-bitcast exp: i16 = int(x*A + B); +0.5 centers the
# truncating float->int conversion.
SCH_A = 128.0 / np.log(2.0)
SCH_B = 127.0 * 128.0 - 0.0301975 * 128.0 + 0.5 + SCH_A * C_BIAS
VSCALE = 0.25     # v'' = v * VSCALE; d column = VSCALE (cancels in o1/r)


def build_program() -> bass.Bass:
    nc = bacc.Bacc(None)

    xt_d = nc.declare_dram_parameter("xt", [KT, 128, N], BF16, False)
    wqk_d = nc.declare_dram_parameter("wqk", [KT, 128, 256], BF16, False)
    wvt_d = nc.declare_dram_parameter("wvt", [KT, 128, 256], BF16, False)
    gb_d = nc.declare_dram_parameter("gb", [4, 128, 256], BF16, False)
    w2s_d = nc.declare_dram_parameter("w2s", [128, 512], BF16, False)
    rstc_d = nc.declare_dram_parameter("rstc", [128, 2], F32, False)
    o1t0_d = nc.declare_dram_parameter("o1t0", [65, N], F16, isOutput=True)
    o1t1_d = nc.declare_dram_parameter("o1t1", [65, N], F16, isOutput=True)
    f2_d = nc.declare_dram_parameter("f2", [NI, 128, 512], F16, isOutput=True)
    o1t_d = [o1t0_d, o1t1_d]

    with tile.TileContext(nc) as tc:
        with (
            tc.tile_pool(name="const", bufs=1) as cp,
            tc.tile_pool(name="at8p", bufs=10) as ap8,
            tc.tile_pool(name="atip", bufs=6) as api,
            tc.tile_pool(name="fout", bufs=4) as fpool,
            tc.tile_pool(name="psum", bufs=1, space="PSUM") as pp,
        ):
            # ---- resident SBUF tensors ----
            xt_sb = cp.tile([128, KT, N], BF16, name="xt_sb")
            wqk_sb = cp.tile([128, KT, 256], BF16, name="wqk_sb")
            wvt_sb = cp.tile([128, KT, 256], BF16, name="wvt_sb")
            g_sb = cp.tile([128, 4, 256], BF16, name="g_sb")
            w2s_sb = cp.tile([128, 512], BF16, name="w2s_sb")
            rstc_sb = cp.tile([128, 2], F32, name="rstc_sb")
            q8tmp = cp.tile([128, N], F8, name="q8tmp")
            k8tmp = cp.tile([128, N], F8, name="k8tmp")
            q8 = cp.tile([64, 2, N], F8, name="q8")
            k8 = cp.tile([64, 2, N], F8, name="k8")
            # v''/d stationary, fp8 DR layout: [p=j, u, jtile, 128] with
            # cols 0:64 = v*VSCALE, col 64 = VSCALE
            vd8 = cp.tile([128, 2, NI, 128], F8, name="vd8")
            # bf16 copy for the DVE-exp route
            vd16 = cp.tile([128, 2, NI, 66], BF16, name="vd16")
            # t' = t/s_j for out2 (cols 0:64 head u0, 64:128 head u1)
            vtt = cp.tile([128, NI, 128], BF16, name="vtt")
            o2sb = cp.tile([128, N], BF16, name="o2sb")
            o1sb = [cp.tile([65, N], F16, name=f"o1sb{u}") for u in range(2)]
            f2sb = cp.tile([128, NI, 512], F16, name="f2sb")

            # warm the ACT exp table off the critical path
            warm = cp.tile([1, 8], F32, name="warm")
            nc.vector.memset(warm[:], 0.0)
            nc.scalar.activation(warm[:], warm[:], EXP)
            # denominator columns (vd16's col 64 comes from the gpsimd copy)
            nc.vector.memset(vd8[:, :, :, 64:65], VSCALE)
            # per-partition exp bias column (scalar consts need an AP)
            cbias = cp.tile([128, 1], F32, name="cbias")
            nc.vector.memset(cbias[:], C_BIAS)

            # ---- input DMAs (critical-path first; one DMA per tensor,
            # xt split in 4 column blocks; no-wait DMAs only on SP) ----
            nc.sync.dma_start(out=xt_sb[:, :, 0:512],
                              in_=xt_d[:, :, 0:512].rearrange("k p c -> p k c"))
            nc.sync.dma_start(out=wqk_sb[:],
                              in_=wqk_d[:].rearrange("k p c -> p k c"))
            for c4 in range(1, 4):
                nc.sync.dma_start(
                    out=xt_sb[:, :, c4 * 512:(c4 + 1) * 512],
                    in_=xt_d[:, :, c4 * 512:(c4 + 1) * 512]
                    .rearrange("k p c -> p k c"))
            nc.sync.dma_start(out=wvt_sb[:],
                              in_=wvt_d[:].rearrange("k p c -> p k c"))
            nc.sync.dma_start(out=g_sb[:],
                              in_=gb_d[:].rearrange("g p f -> p g f"))
            nc.sync.dma_start(out=w2s_sb[:], in_=w2s_d[:])
            nc.sync.dma_start(out=rstc_sb[:], in_=rstc_d[:])

            # ---- emit helpers.  PSUM tags: A0,A1,B0,B1 1 bank; E,F 2 ----
            def emit_qk_chunk(wcol, tmp, c4, tag):
                ps = pp.tile([128, 512], F32, tag=tag, bufs=1, name="qk_ps")
                for kt in range(KT):
                    nc.tensor.matmul(
                        ps, lhsT=wqk_sb[:, kt, wcol:wcol + 128],
                        rhs=xt_sb[:, kt, c4 * 512:(c4 + 1) * 512],
                        start=(kt == 0), stop=(kt == KT - 1))
                nc.vector.tensor_copy(tmp[:, c4 * 512:(c4 + 1) * 512], ps[:])

            def emit_remap(tmp, dst, cs):
                # [dh(128), cols] fp8 -> [32u + dh%32, dh//32, cols]
                # issued on the ACT queue so the SP input-DMA stream never
                # head-of-line blocks on the evac wait
                for u in range(2):
                    for pl in range(2):
                        nc.scalar.dma_start(
                            out=dst[u * 32:(u + 1) * 32, pl, cs],
                            in_=tmp[u * 64 + pl * 32:u * 64 + (pl + 1) * 32,
                                    cs])

            def emit_vt(ip, tag):
                # two j-tiles (2ip, 2ip+1) per 1-bank psum
                ps = pp.tile([128, 2, 256], F32, tag=tag, bufs=1,
                             name="vt_ps")
                for half in range(2):
                    ib = 2 * ip + half
                    for kt in range(KT):
                        nc.tensor.matmul(
                            ps[:, half, :],
                            lhsT=xt_sb[:, kt, ib * 128:(ib + 1) * 128],
                            rhs=wvt_sb[:, kt, :],
                            start=(kt == 0 and half == 0), stop=(kt == KT - 1),
                            skip_group_check=True)
                # t' -> vtt bf16 (1/s_max folded into w2s; only the edge
                # j-tiles 0 and 15 need the per-j correction column)
                if ip == 0:
                    nc.vector.tensor_scalar_mul(
                        vtt[:, 0, :], ps[:, 0, 0:128], rstc_sb[:, 0:1])
                    nc.vector.tensor_copy(vtt[:, 1, :], ps[:, 1, 0:128])
                elif ip == NP - 1:
                    nc.vector.tensor_copy(vtt[:, 14, :], ps[:, 0, 0:128])
                    nc.vector.tensor_scalar_mul(
                        vtt[:, 15, :], ps[:, 1, 0:128], rstc_sb[:, 1:2])
                else:
                    nc.vector.tensor_copy(
                        vtt[:, 2 * ip:2 * ip + 2, :], ps[:, :, 0:128])
                vsrc = ps[:, :, 128:256].rearrange("p i (u f) -> p i u f",
                                                   u=2)
                nc.vector.tensor_copy(
                    vd8[:, :, 2 * ip:2 * ip + 2, 0:64]
                    .rearrange("p u i f -> p i u f"), vsrc)

            at_tiles = {}
            ef_rot = [0]

            def emit_fill(ich, p, u, h):
                """dots psum fill [128 j(2 planes), 512 i] + exp."""
                ef_rot[0] ^= 1
                ps = pp.tile([128, 2, 512], F32,
                             tag=("E" if ef_rot[0] else "Fq"), bufs=1,
                             name="st_ps")
                i0 = ich * 1024 + h * 512
                for pl in range(2):
                    for ic in range(2):
                        nc.tensor.matmul(
                            ps[:, pl, ic * 256:(ic + 1) * 256],
                            lhsT=k8[u * 32:(u + 1) * 32, :,
                                    (p * 2 + pl) * 128:(p * 2 + pl + 1) * 128],
                            rhs=q8[u * 32:(u + 1) * 32, :,
                                   i0 + ic * 256:i0 + (ic + 1) * 256],
                            start=(ic == 0), stop=(ic == 1),
                            perf_mode=DR, skip_group_check=True)
                key = (ich, u, p)
                if key in M_DVE:
                    if key not in at_tiles:
                        at_tiles[key] = api.tile([128, 2, 1024], I16,
                                                 tag="ati", name="ati")
                    at = at_tiles[key]
                    nc.vector.tensor_scalar(
                        out=at[:, :, h * 512:(h + 1) * 512], in0=ps[:],
                        scalar1=SCH_A, scalar2=SCH_B,
                        op0=mybir.AluOpType.mult, op1=mybir.AluOpType.add)
                else:
                    if key not in at_tiles:
                        at_tiles[key] = ap8.tile([128, 2, 1024], F8,
                                                 tag="at8", name="at8")
                    at = at_tiles[key]
                    nc.scalar.activation(
                        at[:, :, h * 512:(h + 1) * 512], ps[:], EXP,
                        bias=cbias[:])

            o1ps = {}

            def emit_out1(ich, u, p):
                key = (ich, u, p)
                at = at_tiles.pop(key)
                tags = ("A0", "A1") if u == 0 else ("B0", "B1")
                for hb in range(2):
                    if (u, ich, hb) not in o1ps:
                        o1ps[(u, ich, hb)] = pp.tile(
                            [65, 512], F32, tag=tags[hb], bufs=1,
                            name=f"o1_ps{u}{hb}")
                if key in M_DVE:
                    for pl in range(2):
                        for hb in range(2):
                            nc.tensor.matmul(
                                o1ps[(u, ich, hb)],
                                lhsT=vd16[:, u, p * 2 + pl, 0:65],
                                rhs=at[:, pl,
                                       hb * 512:(hb + 1) * 512].bitcast(BF16),
                                start=False, stop=(p == NP - 1 and pl == 1),
                                skip_group_check=True)
                else:
                    for hb in range(2):
                        for ic in range(2):
                            nc.tensor.matmul(
                                o1ps[(u, ich, hb)][:, ic * 256:(ic + 1) * 256],
                                lhsT=vd8[:, u, p * 2:p * 2 + 2, 0:65],
                                rhs=at[:, :,
                                       hb * 512 + ic * 256:
                                       hb * 512 + (ic + 1) * 256],
                                start=(p == 0 and ic == 0),
                                stop=(p == NP - 1),
                                perf_mode=DR, skip_group_check=True)

            def emit_o1_dma(ich, u):
                for hb in range(2):
                    ps = o1ps.pop((u, ich, hb))
                    dst = o1sb[u][:, ich * 1024 + hb * 512:
                                  ich * 1024 + (hb + 1) * 512]
                    if hb == 0:
                        nc.vector.tensor_copy(dst, ps[:])
                    else:
                        nc.scalar.copy(dst, ps[:])
                nc.sync.dma_start(
                    out=o1t_d[u][:, ich * 1024:(ich + 1) * 1024],
                    in_=o1sb[u][:, ich * 1024:(ich + 1) * 1024])

            def emit_out2(c, tag):
                ps = pp.tile([128, 256], F32, tag=tag, bufs=1, name="o2_ps")
                jts = [jt for jt in range(2 * c - 1, 2 * c + 3)
                       if 0 <= jt < NI]
                for idx, jt in enumerate(jts):
                    nc.tensor.matmul(
                        ps, lhsT=vtt[:, jt, :],
                        rhs=g_sb[:, jt - 2 * c + 1, :],
                        start=(idx == 0), stop=(idx == len(jts) - 1))
                nc.vector.tensor_copy(o2sb[:, c * 256:(c + 1) * 256], ps[:])

            def emit_f2(ib, tag):
                ps = pp.tile([128, 512], F32, tag=tag, bufs=1, name="f2_ps")
                nc.tensor.matmul(
                    ps, lhsT=o2sb[:, ib * 128:(ib + 1) * 128],
                    rhs=w2s_sb[:], start=True, stop=True)
                if ib % 2 == 0:
                    nc.vector.tensor_copy(f2sb[:, ib, :], ps[:])
                else:
                    nc.scalar.copy(f2sb[:, ib, :], ps[:])
                if ib % 4 == 3:
                    nc.sync.dma_start(
                        out=f2_d[ib - 3:ib + 1].rearrange("i p f -> p i f"),
                        in_=f2sb[:, ib - 3:ib + 1, :])

            # ---- prologue: qk chunks -> fp8 -> remap; early dots fills ----
            emit_qk_chunk(0, k8tmp, 0, "A0")
            emit_remap(k8tmp, k8, slice(0, 512))
            emit_qk_chunk(128, q8tmp, 0, "A1")
            emit_remap(q8tmp, q8, slice(0, 512))
            emit_qk_chunk(0, k8tmp, 1, "A0")
            emit_remap(k8tmp, k8, slice(512, 1024))
            emit_qk_chunk(128, q8tmp, 1, "A1")
            emit_remap(q8tmp, q8, slice(512, 1024))
            for p in range(2):
                for u in range(2):
                    emit_fill(0, p, u, 0)
            emit_qk_chunk(0, k8tmp, 2, "A0")
            emit_qk_chunk(128, q8tmp, 2, "A1")
            for p in range(2):
                for u in range(2):
                    emit_fill(0, p, u, 1)
            emit_qk_chunk(0, k8tmp, 3, "A0")
            emit_remap(k8tmp, k8, slice(1024, 2048))
            emit_qk_chunk(128, q8tmp, 3, "A1")
            emit_remap(q8tmp, q8, slice(1024, 2048))

            # ---- main: remaining ich0 fills + vt + out2 + F2 ----
            b_rot = [0]

            def btag():
                b_rot[0] ^= 1
                return "B0" if b_rot[0] else "B1"

            for p in range(NP):
                emit_vt(p, "A0" if p % 2 == 0 else "A1")
                fp = p + 2
                if fp < NP:
                    for h in range(2):
                        for u in range(2):
                            emit_fill(0, fp, u, h)
                if p >= 1:
                    emit_out2(p - 1, btag())
            # bf16 stationary copy for the DVE-route out1 (Pool engine;
            # sources the fp8 values so both routes see the same v)
            nc.gpsimd.tensor_copy(vd16[:, :, :, 0:65], vd8[:, :, :, 0:65])
            emit_out2(NP - 1, btag())
            for ib in range(NI):
                emit_f2(ib, btag())
            # out1 for ich0 (after F2 so the A/B banks are clear; at tiles
            # buffered in SBUF meanwhile)
            for p in range(NP):
                for u in range(2):
                    emit_out1(0, u, p)
            for u in range(2):
                emit_o1_dma(0, u)
            # ---- ich1 fills + out1 ----
            for p in range(NP):
                for h in range(2):
                    for u in range(2):
                        emit_fill(1, p, u, h)
                if p >= 2:
                    for u in range(2):
                        emit_out1(1, u, p - 2)
            for p in range(NP - 2, NP):
                for u in range(2):
                    emit_out1(1, u, p)
            for u in range(2):
                emit_o1_dma(1, u)

    nc.finalize()
    return nc


_PROGRAM = None


def _get_program():
    global _PROGRAM
    if _PROGRAM is None:
        _PROGRAM = build_program()
    return _PROGRAM


def _host_tables():
    d = np.arange(N, dtype=np.float64)
    g = np.exp(-d / np.e)
    cum = np.cumsum(g)
    j = np.arange(N)
    s = cum[j] + cum[N - 1 - j] - g[0]        # s[j] = sum_k exp(-|j-k|/e)
    smax = s.max()
    # edge corrections s_max/s_j for j-tiles 0 and 15 (elsewhere s_j = s_max
    # to ~1e-10 relative, folded into w2s as the global 1/s_max)
    rstc = np.ascontiguousarray(np.stack(
        [smax / s[0:128], smax / s[N - 128:N]], axis=1).astype(np.float32))
    gi = np.arange(4)[:, None, None]
    p = np.arange(128)[None, :, None]
    f = np.arange(256)[None, None, :]
    gb = np.ascontiguousarray(
        np.exp(-np.abs(f - p - (gi - 1) * 128) / np.e)
        .astype(ml_dtypes.bfloat16))
    return rstc, gb, smax


_TABLES = None


def _tables():
    global _TABLES
    if _TABLES is None:
        _TABLES = _host_tables()
    return _TABLES


def make_in_maps(x, w_qkv, w_out, b_out):
    x = np.asarray(x, np.float32)
    w_qkv = np.asarray(w_qkv, np.float32)
    w_out = np.asarray(w_out, np.float32)
    rstc, gb, smax = _tables()
    qk_scale = float(DH) ** -0.25        # 1/sqrt(8) folded into both w's

    wq_full = w_qkv[0:512]
    wk_full = w_qkv[512:1024]
    wv_full = w_qkv[1024:1536]
    wt_full = w_qkv[1536:2048]

    bf = ml_dtypes.bfloat16
    in_maps = []
    for c in range(NCORES):
        b = c // 4
        h0 = 2 * (c % 4)
        h1 = h0 + 1

        xt = np.ascontiguousarray(x[b].T.reshape(KT, 128, N).astype(bf))

        def pack2(wfull, scl):
            wt_ = np.concatenate(
                [wfull[h0 * 64:(h0 + 1) * 64].T * scl,
                 wfull[h1 * 64:(h1 + 1) * 64].T * scl], axis=1)
            return np.ascontiguousarray(
                wt_.reshape(KT, 128, 128).astype(bf))

        wqk_ = np.concatenate(
            [wk_full[h0 * 64:(h0 + 1) * 64].T * qk_scale,
             wk_full[h1 * 64:(h1 + 1) * 64].T * qk_scale,
             wq_full[h0 * 64:(h0 + 1) * 64].T * qk_scale,
             wq_full[h1 * 64:(h1 + 1) * 64].T * qk_scale], axis=1)
        wqk = np.ascontiguousarray(wqk_.reshape(KT, 128, 256).astype(bf))
        wvt_ = np.concatenate(
            [wt_full[h0 * 64:(h0 + 1) * 64].T,
             wt_full[h1 * 64:(h1 + 1) * 64].T,
             wv_full[h0 * 64:(h0 + 1) * 64].T * VSCALE,
             wv_full[h1 * 64:(h1 + 1) * 64].T * VSCALE], axis=1)
        wvt = np.ascontiguousarray(wvt_.reshape(KT, 128, 256).astype(bf))
        w2s = np.ascontiguousarray((np.concatenate(
            [w_out[:, h0 * 128 + 64:(h0 + 1) * 128].T,
             w_out[:, h1 * 128 + 64:(h1 + 1) * 128].T],
            axis=0) / smax).astype(bf))
        in_maps.append({
            "xt": xt, "wqk": wqk, "wvt": wvt,
            "gb": gb, "w2s": w2s, "rstc": rstc,
        })
    return in_maps


def combine_outputs(results, w_out, b_out):
    """Host-side unshard: per-core partials -> full [B, N, DIM] output."""
    w_out = np.asarray(w_out, np.float32)
    b_out = np.asarray(b_out, np.float32)
    out = np.zeros((B, N, 512), np.float64)
    for c in range(NCORES):
        r = results[c]
        b = c // 4
        h0 = 2 * (c % 4)
        part = r["f2"].reshape(N, 512).astype(np.float64)
        for u, h in ((0, h0), (1, h0 + 1)):
            o1 = r[f"o1t{u}"].astype(np.float64)
            o1v = (o1[0:64] / o1[64][None, :]).T            # [N, 64]
            w1 = w_out[:, h * 128:h * 128 + 64].T.astype(np.float64)
            part = part + o1v @ w1
        out[b] += part
    out += b_out[None, None, :].astype(np.float64)
    return out.astype(np.float32)


def kernel(x, w_qkv, w_out, b_out):
    nc = _get_program()
    in_maps = make_in_maps(x, w_qkv, w_out, b_out)
    res = run_bass_kernel_spmd(nc, in_maps, core_ids=list(range(NCORES)))
    return combine_outputs(res.results, w_out, b_out)


def kernel_profiled(x, w_qkv, w_out, b_out):
    out = kernel(x, w_qkv, w_out, b_out)
    return out, None
